# revision 27
# baseline (speedup 1.0000x reference)
"""Paged causal GQA attention (prefill) on 8 TRN2 NeuronCores.

Sharding: tensor-parallel over heads. Core c computes heads {2c, 2c+1},
which share KV head c//2 (GQA group size 4). No collectives needed.

Host side does the paged-cache store + block-table gather (pure indexing),
casts Q/K/V to fp16, pre-transposes Q/K to [d, seq] layout and prepacks
V/output layouts so every device DMA is a large contiguous transfer
(no xbar DMA-transposes, no small-descriptor gathers).

Per-core device kernel (fp16 matmuls, f32 PSUM accumulate):
  - kT/qT loaded directly [d=128, seq] fp16 (host pre-transposed);
    kT + V + output stores on the Sync HWDGE ring, qT on the Scalar
    ring; startup-critical chunks are split so the first QK can start
    as soon as ~96KB have landed (~10us; the ~7.4us before that is
    fixed framework preamble)
  - V loaded [k, d] fp16 with a ones-column appended, so the softmax
    denominator comes out of the same PV matmul (column 128)
  - S^T tiles = kT_i^T @ qT (PSUM f32), causally trimmed per k-tile
  - exp is SPLIT across two engines: ScalarE ACTIVATE(Exp) and VectorE
    via a Schraudolph-style bit-trick exp (i16 = s*C1 + C2 in one
    TENSOR_SCALAR, bitcast to fp16 ~= exp(s*SCALE); ~3% max err on
    those tiles, washes out in the softmax average).  The V/A pattern
    (_ENG_PLAN) alternates in stream order and is load-balanced so
    ACT ~= DVE + norm work (~13.2us/head each, under PE's 15us/head);
    only J=0 (few-key denominators) is pinned to the table exp
  - triangular 0/1 mask multiply on diagonal 128x128 blocks runs on
    the (otherwise idle) GpSimd engine, off the DVE critical path
  - three-stage software pipeline: QK(n) | exp+mask(n-2) | PV(n-4),
    so the in-order PE stream always has masked P ready when the PV
    matmuls issue, and exp engines always have S^T ready; PSUM-bank
    norms are emitted directly at group completion (the pipeline lag
    plus the deferred bank-1 writes provide the reuse slack)
  - PV = PT^T @ V_aug accumulated in PSUM; per-bank batched reciprocal
    + one broadcast multiply per bank into the fp32 output (VectorE)
  - 11 warmup matmuls bridge the PE from the preamble end to the first
    data arrival so the HAM clock gate (1.2 -> 2.4 GHz after ~3.4us of
    SUSTAINED activity) trips right as real work starts; any idle gap
    here re-throttles the clock and costs ~2x on everything cold

Measured: ~146-150us (run-to-run DVFS/HAM-phase variance ~ +/-2us) vs
~161-162us for the previous baseline; rel_inf err 4.7e-3 (gate 2e-2).
PE array busy ~129us of a ~133us stream window (fp16 streaming floor
for the causal QK+PV column count is ~116.5us; fp8 DoubleRow cannot
help: QK contraction is 128 and the q-stationary PV that carries the
free denominator column has FD=129, below the DoubleRow win point).

CAUTION: the startup DMA emission order/count is load-bearing.  DMA
completion semaphores come from a small compile-time round-robin pool;
adding or reordering early dma_start calls reshuffles the assignment
and can make a startup-critical chunk wait multi-us on an unrelated
transfer's semaphore (measured +5us and +32us on two such attempts).
"""

import os
import sys

import numpy as np

sys.path.insert(0, "/opt/trn_rl_repo")

T, H, HKV, D = 8192, 16, 4, 128
NB, BS = 64, 256
B, BPS = 4, 8
S = BPS * BS  # 2048 per-sequence length
NCORES = 8
HPC = H // NCORES  # heads per core = 2
SCALE = 0.08838834764831845
NT = S // 128  # 16 key tiles (and query tiles) per sequence
QG = 512  # query-group width for the QK matmul
NG = S // QG  # 4 query groups
EB = 2  # k-tiles per exp batch

# Schraudolph fp16 exp: bits(i16) = round(s*C1 + C2); bitcast -> ~exp(s*SCALE)
LOG2E = 1.4426950408889634
C1 = SCALE * LOG2E * 1024.0
C2 = 15360.0 - 44.0

_cache = {}

LAST_RESULTS = None  # stash of the most recent BassKernelResults (for profiling)

# per-J engine pattern for ALL batches in stream order: 'V' = VectorE
# bit-trick, 'A' = ScalarE ACTIVATE.  J=0 is pinned to 'A' (its queries
# have few-key denominators, so the ~3% bit-trick sawtooth wouldn't
# average out); everything else alternates so neither exp engine ever
# runs more than ~2 batches back-to-back, keeping head boundaries from
# serializing on ScalarE.  V count is tuned so ACT ~= DVE + norms.
_ENG_PLAN = {
    0: ["A", "A"],
    1: ["V", "A", "V", "A"],
    2: ["V", "A", "A", "V", "A", "A"],
    3: ["V", "A", "A", "V", "A", "A", "V", "A"],
}


def _group_plan(J, last_head):
    """Exp batches for query-group J: list of (k_tiles, qoff, eng).
    K-tiles up to and including the first diagonal pair go in batches of
    EB; the second diagonal pair only sees queries >= 256 of the group so
    it is q-sliced into its own batch."""
    nd = 4 * J + 2
    plan = []
    i = 0
    k = 0
    while i < nd:
        sz = min(EB, nd - i)
        plan.append((list(range(i, i + sz)), 0, _ENG_PLAN[J][k]))
        i += sz
        k += 1
    plan.append(([nd, nd + 1], 256, _ENG_PLAN[J][-1]))
    return plan


def _build_nc():
    import concourse.bass as bass
    import concourse.tile as tile
    from concourse import bacc, mybir

    ts = bass.ts
    f32, f16 = mybir.dt.float32, mybir.dt.float16
    i16 = mybir.dt.int16
    Exp = mybir.ActivationFunctionType.Exp
    mult = mybir.AluOpType.mult
    addop = mybir.AluOpType.add

    nc = bacc.Bacc(
        "TRN2",
        target_bir_lowering=False,
        debug=False,
        enable_asserts=False,
        num_devices=NCORES,
    )
    q_in = nc.dram_tensor("q", [B, HPC, D, S], f16, kind="ExternalInput").ap()
    k_in = nc.dram_tensor("k", [B, D, S], f16, kind="ExternalInput").ap()
    # v prepacked on host as [p=s%128, t=s//128, d] for contiguous DMA
    v_in = nc.dram_tensor("v", [B, 128, NT, D], f16, kind="ExternalInput").ap()
    tri_in = nc.dram_tensor("tri", [128, 128], f16, kind="ExternalInput").ap()
    # out in [h, p, t, d] layout (host transposes back) for contiguous DMA
    out = nc.dram_tensor("out", [B, HPC, 128, NT, D], f16, kind="ExternalOutput").ap()

    with tile.TileContext(nc) as tc:
        with (
            tc.tile_pool(name="kv", bufs=1) as kvpool,
            tc.tile_pool(name="qt", bufs=3) as qpool,
            tc.tile_pool(name="pt", bufs=8) as ptpool,
            tc.tile_pool(name="ob", bufs=3) as opool,
            tc.tile_pool(name="sm", bufs=8) as smpool,
            tc.tile_pool(name="ps_s", bufs=3, space="PSUM") as pspool,
            tc.tile_pool(name="ps_o", bufs=2, space="PSUM") as popool,
        ):
            # dummy exp up front: pulls the ScalarE exp table load (~2.7us)
            # off the first real ACTIVATE's critical path
            warm = kvpool.tile([128, 1], f32, tag="warm")
            nc.vector.memset(warm[:], 0.0)
            nc.scalar.activation(
                warm[:], warm[:], mybir.ActivationFunctionType.Exp, scale=1.0
            )

            # dummy matmul chain: ramps the PE HAM clock gate (1.2 -> 2.4 GHz
            # after ~3.4us of continuous work) while the first kT/qT DMAs are
            # still in flight; sized so it ends right as the first inputs
            # land (~10.5us) and the HAM window trips
            wlhs = kvpool.tile([128, 128], f16, tag="wlhs")
            wrhs = kvpool.tile([128, 256], f16, tag="wrhs")
            nc.vector.memset(wlhs[:], 0.0)
            nc.vector.memset(wrhs[:], 0.0)
            wps = pspool.tile([128, EB, QG], f32, tag="ps", name="warmps")
            for _ in range(12):
                nc.tensor.matmul(
                    wps[:, 0, 0:256], lhsT=wlhs[:], rhs=wrhs[:], start=True, stop=True
                )

            from collections import deque

            tri = kvpool.tile([128, 128], f16, tag="tri")
            kT = {}
            vaug = {}

            def _prep_b(b):
                kT_b = kvpool.tile([128, S], f16, tag=f"kT{b}", name=f"kT{b}")
                va = kvpool.tile([128, NT, 132], f16, tag=f"va{b}", name=f"va{b}")
                if b == 0:
                    # startup chunking: each group J's kT/va arrives just
                    # before its QK/PV batches need it, so the first real
                    # matmuls start as soon as the ring delivers ~96KB
                    nc.sync.dma_start(out=kT_b[:, 0:256], in_=k_in[b][:, 0:256])
                    nc.sync.dma_start(out=kT_b[:, 256:768], in_=k_in[b][:, 256:768])
                    nc.sync.dma_start(out=tri[:], in_=tri_in)
                    nc.sync.dma_start(out=va[:, 0:4, 0:128], in_=v_in[b][:, 0:4])
                    nc.sync.dma_start(out=kT_b[:, 768:S], in_=k_in[b][:, 768:S])
                    nc.sync.dma_start(out=va[:, 4:8, 0:128], in_=v_in[b][:, 4:8])
                    nc.sync.dma_start(out=va[:, 8:NT, 0:128], in_=v_in[b][:, 8:NT])
                else:
                    nc.sync.dma_start(out=kT_b[:], in_=k_in[b])
                    nc.sync.dma_start(out=va[:, :, 0:128], in_=v_in[b])
                kT[b] = kT_b
                nc.vector.memset(va[:, :, 128:129], 1.0)
                vaug[b] = va

            class Ctx:
                def __init__(self, b, h):
                    self.b, self.h = b, h
                    qT = qpool.tile([128, S], f16, tag="qT", name=f"qT{b}_{h}")
                    if (b, h) == (0, 0):
                        # first qT chunked on the Scalar ring so it loads in
                        # parallel with kT0 on the Sync ring (startup path)
                        nc.scalar.dma_start(out=qT[:, 0:QG], in_=q_in[b, h][:, 0:QG])
                        nc.scalar.dma_start(
                            out=qT[:, QG : 2 * QG], in_=q_in[b, h][:, QG : 2 * QG]
                        )
                        nc.scalar.dma_start(
                            out=qT[:, 2 * QG : S], in_=q_in[b, h][:, 2 * QG : S]
                        )
                    else:
                        nc.scalar.dma_start(out=qT[:], in_=q_in[b, h])
                    self.qT = qT
                    self.ob = opool.tile([128, NT, D], f16, tag="ob", name=f"ob{b}_{h}")
                    self.po_of = {}
                    self.defer = deque()  # deferred r>=2 PV batches
                    self.done_groups = 0
                    self.last = (b, h) == (B - 1, HPC - 1)
                    # reverse the group order on the final head so the tail
                    # after the last exp is the smallest group's work
                    Js = range(NG - 1, -1, -1) if self.last else range(NG)
                    self.batches = [
                        (J, ktl, qoff, eng)
                        for J in Js
                        for (ktl, qoff, eng) in _group_plan(J, self.last)
                    ]

                def norm_bank(self, J, x):
                    # batched: one reciprocal per po bank (2 q-rows), then a
                    # single broadcast multiply into the fp32 output tile
                    po = self.po_of[J][x]
                    linv = smpool.tile([128, 2, 1], f32, tag="linv", name="linv")
                    nc.vector.reciprocal(linv[:], po[:, 0:2, 128:129])
                    nc.vector.tensor_tensor(
                        self.ob[:, 4 * J + 2 * x : 4 * J + 2 * x + 2, :],
                        po[:, 0:2, 0:128],
                        linv[:].broadcast_to([128, 2, 128]),
                        mult,
                    )
                    if self.last:
                        # per-group stores (per-bank for the final group) so
                        # the tail's last DMA is small but store-issue time
                        # on the Sync queue stays off the critical path
                        if J == 0:
                            lo = 2 * x
                            nc.sync.dma_start(
                                out=out[self.b, self.h][:, lo : lo + 2, :],
                                in_=self.ob[:, lo : lo + 2, :],
                            )
                        elif x == 1:
                            nc.sync.dma_start(
                                out=out[self.b, self.h][:, 4 * J : 4 * J + 4, :],
                                in_=self.ob[:, 4 * J : 4 * J + 4, :],
                            )

                def emit_qk(self, J, ktl, qoff):
                    qw = QG - qoff
                    ps = pspool.tile([128, EB, qw], f32, tag="ps", name="ps")
                    pt = ptpool.tile([128, EB, qw], f16, tag="pt", name="pt")
                    for u, iu in enumerate(ktl):
                        # causal trim: k-tile iu only matters for queries
                        # >= 128*(iu-4J); the skipped region holds stale PSUM
                        # that exp bounds and PV never reads
                        qo = max(qoff, 128 * (iu - 4 * J))
                        nc.tensor.matmul(
                            ps[:, u, qo - qoff : qw],
                            lhsT=kT[self.b][:, ts(iu, 128)],
                            rhs=self.qT[:, J * QG + qo : (J + 1) * QG],
                            start=True,
                            stop=True,
                        )
                    return ps, pt

                def emit_exp(self, J, ktl, qoff, eng, ps, pt):
                    # stage 1: exp on the planned engine, then triangular
                    # masks for any diagonal tiles on the GpSimd engine
                    # (keeps DVE free and mask latency off the PV path)
                    nu = len(ktl)
                    if eng == "V":
                        nc.vector.tensor_scalar(
                            pt[:, 0:nu, :].bitcast(i16),
                            ps[:, 0:nu, :],
                            C1,
                            C2,
                            mult,
                            addop,
                        )
                    else:
                        nc.scalar.activation(
                            pt[:, 0:nu, :], ps[:, 0:nu, :], Exp, scale=SCALE
                        )
                    for u, iu in enumerate(ktl):
                        rp = iu - 4 * J
                        if rp >= 0:
                            lo = 128 * rp - qoff
                            nc.gpsimd.tensor_tensor(
                                pt[:, u, lo : lo + 128],
                                pt[:, u, lo : lo + 128],
                                tri[:],
                                mult,
                            )

                def emit_pv_stage(self, J, ktl, qoff, eng, ps, pt):
                    # stage 2: PV matmuls, PSUM-bank norms, stores
                    if J not in self.po_of:
                        # two packed PV accumulators: (r=0,1) and (r=2,3).
                        # The very last group's bank-1 comes out of the (by
                        # now draining) QK-score pool so the kernel tail does
                        # not serialize on the 2-buffer po rotation.
                        def _po(x):
                            if self.last and J == 0 and x == 1:
                                return pspool.tile(
                                    [128, 2, 132], f32, tag="ps", name="potail"
                                )
                            return popool.tile(
                                [128, 2, 132],
                                f32,
                                tag="po",
                                name=f"po{self.b}{self.h}{J}{x}",
                            )

                        self.po_of[J] = [_po(0), _po(1)]
                    # batches without diagonal tiles, plus J=0's diagonal
                    # batch (the head's first), defer their bank-1 PVs so the
                    # first bank-1 writes of a group land well after the
                    # previous group's bank-1 norm has freed the bank
                    if ktl[-1] < 4 * J or (J == 0 and ktl[-1] == 1):
                        for u, iu in enumerate(ktl):
                            self.emit_pv(J, qoff, pt, u, iu, 0, 2)
                            if iu - 4 * J == 1:
                                # bank 0 (r=0,1) completes with this batch:
                                # normalize early so it frees for the next
                                # group
                                self.norm_bank(J, 0)
                        self.defer.append((J, qoff, pt, ktl))
                        while len(self.defer) > 2:
                            self.flush_defer()
                        return
                    # diagonal/pair batch: flush this group's deferred bank-1
                    # accumulations, which must precede the immediate PVs
                    while self.defer:
                        self.flush_defer()
                    for u, iu in enumerate(ktl):
                        rp = iu - 4 * J
                        self.emit_pv(J, qoff, pt, u, iu, 0, 4)
                        if rp == 1:
                            self.norm_bank(J, 0)
                    if iu == 4 * J + 3:  # last batch of the group
                        self.norm_bank(J, 1)
                        self.done_groups += 1
                        self.store(J)

                def emit_pv(self, J, qoff, pt, u, iu, rlo, rhi):
                    rp = iu - 4 * J
                    po = self.po_of[J]
                    for r in range(max(rp, rlo), rhi):
                        # start=True clears has_written for the WHOLE bank;
                        # only the bank's first group (even r) may set it.
                        lo = 128 * r - qoff
                        nc.tensor.matmul(
                            po[r // 2][:, r % 2, 0:129],
                            lhsT=pt[:, u, lo : lo + 128],
                            rhs=vaug[self.b][:, iu, 0:129],
                            start=(iu == 0 and r % 2 == 0),
                            stop=(iu == 4 * J + r),
                        )

                def flush_defer(self):
                    J, qoff, pt, ktl = self.defer.popleft()
                    for u, iu in enumerate(ktl):
                        self.emit_pv(J, qoff, pt, u, iu, 2, 4)

                def store(self, J):
                    # stores go out the Sync ring (with kT/va loads), keeping
                    # the Scalar ring free for qT prefetches
                    if not self.last and self.done_groups == NG:
                        nc.sync.dma_start(
                            out=out[self.b, self.h], in_=self.ob[:]
                        )

            # one flat software-pipelined stream across all (b, h) with two
            # lag stages: batch n's QK matmuls are emitted at step n, its
            # exp+masks at step n+2 and its PV matmuls at step n+4, so the
            # in-order PE stream always has masked P ready when the PV
            # matmuls issue, including across head and sequence boundaries.
            heads = [(b, h) for b in range(B) for h in range(HPC)]
            _prep_b(0)
            pend = deque()  # QK emitted, awaiting exp+mask
            pend2 = deque()  # exp emitted, awaiting PV
            next_ctx = Ctx(*heads[0])

            def s1():
                item = pend.popleft()
                item[0].emit_exp(*item[1:])
                pend2.append(item)

            def s2():
                item = pend2.popleft()
                item[0].emit_pv_stage(*item[1:])

            for idx, (b, h) in enumerate(heads):
                ctx = next_ctx
                next_ctx = None
                if h == 0 and b + 1 < B:
                    _prep_b(b + 1)
                nbat = len(ctx.batches)
                for k, bt in enumerate(ctx.batches):
                    if nbat - k == 6 and idx + 1 < len(heads):
                        next_ctx = Ctx(*heads[idx + 1])
                    # stage-2 PVs go on the PE queue BEFORE this step's QK:
                    # the QK may wait on ps-buffer reuse (exp 3 batches back)
                    # at the queue head, and ready PV work must not be stuck
                    # behind that wait
                    if len(pend2) > 2:
                        s2()
                    eb = ctx.emit_qk(bt[0], bt[1], bt[2])
                    pend.append((ctx, bt[0], bt[1], bt[2], bt[3], eb[0], eb[1]))
                    if len(pend) > 2:
                        s1()
                if next_ctx is None and idx + 1 < len(heads):
                    next_ctx = Ctx(*heads[idx + 1])
            while pend:
                if pend2:
                    s2()
                s1()
            while pend2:
                s2()
    nc.compile()
    return nc


def _get_nc():
    if "nc" not in _cache:
        _cache["nc"] = _build_nc()
    return _cache["nc"]


def _install_ntff_hook():
    """Register the axon NTFF profile hook that concourse expects under
    ``antenv.axon_hooks`` (the agent image lacks that module). Mirrors
    trn_agent_boot's ctypes shim. Returns True if profiling is available."""
    import contextlib
    import ctypes
    import types

    if "antenv.axon_hooks" in sys.modules:
        return True
    so_path = "/opt/axon/libaxon_pjrt.so"
    if not os.path.exists(so_path):
        return False
    lib = ctypes.CDLL(so_path)
    if not hasattr(lib, "axon_start_nrt_profile"):
        return False
    lib.axon_start_nrt_profile.argtypes = [
        ctypes.POINTER(ctypes.c_int64),
        ctypes.c_size_t,
    ]
    lib.axon_start_nrt_profile.restype = ctypes.c_int64
    lib.axon_stop_nrt_profile.argtypes = [ctypes.c_char_p]
    lib.axon_stop_nrt_profile.restype = ctypes.c_int64

    @contextlib.contextmanager
    def _hook(output_dir, device_ids):
        import jax

        jax.devices()
        if device_ids:
            ids = (ctypes.c_int64 * len(device_ids))(*device_ids)
            rc = lib.axon_start_nrt_profile(ids, len(device_ids))
        else:
            rc = lib.axon_start_nrt_profile(None, 0)
        if rc != 0:
            raise RuntimeError(f"axon_start_nrt_profile rc={rc}")
        try:
            yield
        finally:
            n = lib.axon_stop_nrt_profile(str(output_dir).encode())
            print(f"ntff profile: {n} file(s) -> {output_dir}", file=sys.stderr)

    import antenv

    mod = types.ModuleType("antenv.axon_hooks")
    _h = [_hook]
    mod.get_axon_ntff_profile_hook = lambda: _h[0]
    mod.set_axon_ntff_profile_hook = lambda h: _h.__setitem__(0, h)
    sys.modules["antenv.axon_hooks"] = mod
    antenv.axon_hooks = mod

    # keep the trace path local: no artifact upload from this container
    from concourse import bass_utils as _bu

    _bu.upload_artifacts = lambda d: f"file://{d}"
    return True


def kernel(q, k, v, k_cache, v_cache, slot_mapping, block_tables):
    global LAST_RESULTS
    from concourse.bass_utils import run_bass_kernel_spmd

    q = np.ascontiguousarray(np.asarray(q), dtype=np.float32)
    k = np.ascontiguousarray(np.asarray(k), dtype=np.float32)
    v = np.ascontiguousarray(np.asarray(v), dtype=np.float32)
    sm = np.asarray(slot_mapping).astype(np.int64)
    bt = np.asarray(block_tables).astype(np.int64)

    # paged KV-cache store + gather through block tables (host side: pure
    # data movement, mirrors the reference semantics incl. dropped slots)
    num_slots = NB * BS
    kc = np.asarray(k_cache, dtype=np.float32).reshape(num_slots, HKV, D).copy()
    vc = np.asarray(v_cache, dtype=np.float32).reshape(num_slots, HKV, D).copy()
    valid = (sm >= 0) & (sm < num_slots)
    kc[sm[valid]] = k[valid]
    vc[sm[valid]] = v[valid]
    btc = np.clip(bt, 0, NB - 1)  # jax gather clamps OOB indices
    k_seq = kc.reshape(NB, BS, HKV, D)[btc].reshape(B, S, HKV, D)
    v_seq = vc.reshape(NB, BS, HKV, D)[btc].reshape(B, S, HKV, D)

    # pre-transpose to [d, seq] so device DMA loads are plain contiguous
    q16 = q.reshape(B, S, H, D).astype(np.float16)
    qT = np.ascontiguousarray(q16.transpose(0, 2, 3, 1))  # [B, H, D, S]
    kT = np.ascontiguousarray(
        k_seq.astype(np.float16).transpose(0, 2, 3, 1)
    )  # [B, HKV, D, S]
    # v prepacked to [B, HKV, p, t, d] (s = t*128 + p) for contiguous DMA
    v16 = np.ascontiguousarray(
        v_seq.astype(np.float16).reshape(B, NT, 128, HKV, D).transpose(0, 3, 2, 1, 4)
    )
    tri = np.triu(np.ones((128, 128), dtype=np.float16))

    in_maps = []
    for c in range(NCORES):
        g = c // 2  # this core's KV head
        in_maps.append(
            {
                "q": np.ascontiguousarray(qT[:, HPC * c : HPC * (c + 1)]),
                "k": np.ascontiguousarray(kT[:, g]),
                "v": np.ascontiguousarray(v16[:, g]),
                "tri": tri,
            }
        )

    nc = _get_nc()
    trace = bool(int(os.environ.get("KERNEL_TRACE", "0")))
    if trace:
        trace = _install_ntff_hook()
    tmpdir = os.environ.get("KERNEL_TRACE_DIR") or None
    if tmpdir:
        os.makedirs(tmpdir, exist_ok=True)
    res = run_bass_kernel_spmd(
        nc, in_maps, core_ids=list(range(NCORES)), trace=trace, tmpdir=tmpdir
    )
    LAST_RESULTS = res

    out = np.empty((B, S, H, D), np.float32)
    for c in range(NCORES):
        # device layout [B, HPC, p, t, d] -> [B, s=t*128+p, HPC, d]
        r = res.results[c]["out"].astype(np.float32).transpose(0, 3, 2, 1, 4).reshape(B, S, HPC, D)
        out[:, :, HPC * c : HPC * (c + 1), :] = r
    return out.reshape(T, H, D)


# revision 30
# speedup vs baseline: 1.0120x; 1.0120x over previous
"""Paged causal GQA attention (prefill) on 8 TRN2 NeuronCores.

Sharding: tensor-parallel over heads. Core c computes heads {2c, 2c+1},
which share KV head c//2 (GQA group size 4). No collectives needed.

Host side does the paged-cache store + block-table gather (pure indexing),
casts Q/K/V to fp16, pre-transposes Q/K to [d, seq] layout and prepacks
V/output layouts so every device DMA is a large contiguous transfer
(no xbar DMA-transposes, no small-descriptor gathers).

Per-core device kernel (fp16 matmuls, f32 PSUM accumulate):
  - kT/qT loaded directly [d=128, seq] fp16 (host pre-transposed);
    kT + V + output stores on the Sync HWDGE ring, qT on the Scalar
    ring; startup-critical chunks are split so the first QK can start
    as soon as ~96KB have landed (~10us; the ~7.4us before that is
    fixed framework preamble)
  - V loaded [k, d] fp16 with a ones-column appended, so the softmax
    denominator comes out of the same PV matmul (column 128)
  - S^T tiles = kT_i^T @ qT (PSUM f32), causally trimmed per k-tile
  - exp is SPLIT across two engines: ScalarE ACTIVATE(Exp) and VectorE
    via a Schraudolph-style bit-trick exp (i16 = s*C1 + C2 in one
    TENSOR_SCALAR, bitcast to fp16 ~= exp(s*SCALE); ~3% max err on
    those tiles, washes out in the softmax average).  The V/A pattern
    (_ENG_PLAN) alternates in stream order and is load-balanced so
    ACT ~= DVE + norm work (~13.2us/head each, under PE's 15us/head);
    only J=0 (few-key denominators) is pinned to the table exp
  - triangular 0/1 mask multiply on diagonal 128x128 blocks runs on
    the (otherwise idle) GpSimd engine, off the DVE critical path
  - three-stage software pipeline: QK(n) | exp+mask(n-2) | PV(n-4),
    so the in-order PE stream always has masked P ready when the PV
    matmuls issue, and exp engines always have S^T ready; PSUM-bank
    norms are emitted directly at group completion (the pipeline lag
    plus the deferred bank-1 writes provide the reuse slack)
  - PV = PT^T @ V_aug accumulated in PSUM; per-bank batched reciprocal
    + one broadcast multiply per bank into the fp32 output (VectorE)
  - 11 warmup matmuls bridge the PE from the preamble end to the first
    data arrival so the HAM clock gate (1.2 -> 2.4 GHz after ~3.4us of
    SUSTAINED activity) trips right as real work starts; any idle gap
    here re-throttles the clock and costs ~2x on everything cold

Measured: ~146.5us (fast clock state; 4 samples 146.1-146.6) vs
~161-162us for the previous baseline; rel_inf err 4.7e-3 (gate 2e-2).
PE array busy ~129us of a ~133us stream window (fp16 streaming floor
for the causal QK+PV column count is ~116.5us; fp8 DoubleRow cannot
help: QK contraction is 128 and the q-stationary PV that carries the
free denominator column has FD=129, below the DoubleRow win point).

CAUTION 1: the part is bimodal -- under chip-level power throttling
(P0, ~2.0 GHz PLL) ALL engines run ~1.20x slower and the same binary
measures ~178us.  Never A/B a change on single runs; check engine
busy-time ratios in the trace to identify the clock state first.

CAUTION 2: the startup DMA emission order/count is load-bearing.  DMA
completion semaphores come from a small compile-time round-robin pool;
adding or reordering early dma_start calls reshuffles the assignment
and can make a startup-critical chunk wait multi-us on an unrelated
transfer's semaphore (measured +5us on one such attempt: the tri load
blocked on a qT chunk's recycled semaphore, delaying the first masks).

Tried and rejected (measured, fast-state): emitting the stage-2 PVs
before each step's QK on the PE queue (+1.8us -- the early QK fills
buy more overlap than the head-of-line wait costs); exp batches for
the final diag pair on VectorE (no gain); extra warmup MMs past the
data-arrival point (delays real work 1:1).
"""

import os
import sys

import numpy as np

sys.path.insert(0, "/opt/trn_rl_repo")

T, H, HKV, D = 8192, 16, 4, 128
NB, BS = 64, 256
B, BPS = 4, 8
S = BPS * BS  # 2048 per-sequence length
NCORES = 8
HPC = H // NCORES  # heads per core = 2
SCALE = 0.08838834764831845
NT = S // 128  # 16 key tiles (and query tiles) per sequence
QG = 512  # query-group width for the QK matmul
NG = S // QG  # 4 query groups
EB = 2  # k-tiles per exp batch

# Schraudolph fp16 exp: bits(i16) = round(s*C1 + C2); bitcast -> ~exp(s*SCALE)
LOG2E = 1.4426950408889634
C1 = SCALE * LOG2E * 1024.0
C2 = 15360.0 - 44.0

_cache = {}

LAST_RESULTS = None  # stash of the most recent BassKernelResults (for profiling)

# per-J engine pattern for ALL batches in stream order: 'V' = VectorE
# bit-trick, 'A' = ScalarE ACTIVATE.  J=0 is pinned to 'A' (its queries
# have few-key denominators, so the ~3% bit-trick sawtooth wouldn't
# average out); everything else alternates so neither exp engine ever
# runs more than ~2 batches back-to-back, keeping head boundaries from
# serializing on ScalarE.  V count is tuned so ACT ~= DVE + norms.
_ENG_PLAN = {
    0: ["A", "A"],
    1: ["V", "A", "V", "A"],
    2: ["V", "A", "A", "V", "A", "A"],
    3: ["V", "A", "A", "V", "A", "A", "V", "A"],
}


def _group_plan(J, last_head):
    """Exp batches for query-group J: list of (k_tiles, qoff, eng).
    K-tiles up to and including the first diagonal pair go in batches of
    EB; the second diagonal pair only sees queries >= 256 of the group so
    it is q-sliced into its own batch."""
    nd = 4 * J + 2
    plan = []
    i = 0
    k = 0
    while i < nd:
        sz = min(EB, nd - i)
        plan.append((list(range(i, i + sz)), 0, _ENG_PLAN[J][k]))
        i += sz
        k += 1
    plan.append(([nd, nd + 1], 256, _ENG_PLAN[J][-1]))
    return plan


def _build_nc():
    import concourse.bass as bass
    import concourse.tile as tile
    from concourse import bacc, mybir

    ts = bass.ts
    f32, f16 = mybir.dt.float32, mybir.dt.float16
    i16 = mybir.dt.int16
    Exp = mybir.ActivationFunctionType.Exp
    mult = mybir.AluOpType.mult
    addop = mybir.AluOpType.add

    nc = bacc.Bacc(
        "TRN2",
        target_bir_lowering=False,
        debug=False,
        enable_asserts=False,
        num_devices=NCORES,
    )
    q_in = nc.dram_tensor("q", [B, HPC, D, S], f16, kind="ExternalInput").ap()
    k_in = nc.dram_tensor("k", [B, D, S], f16, kind="ExternalInput").ap()
    # v prepacked on host as [p=s%128, t=s//128, d] for contiguous DMA
    v_in = nc.dram_tensor("v", [B, 128, NT, D], f16, kind="ExternalInput").ap()
    tri_in = nc.dram_tensor("tri", [128, 128], f16, kind="ExternalInput").ap()
    # out in [h, p, t, d] layout (host transposes back) for contiguous DMA
    out = nc.dram_tensor("out", [B, HPC, 128, NT, D], f16, kind="ExternalOutput").ap()

    with tile.TileContext(nc) as tc:
        with (
            tc.tile_pool(name="kv", bufs=1) as kvpool,
            tc.tile_pool(name="qt", bufs=3) as qpool,
            tc.tile_pool(name="pt", bufs=8) as ptpool,
            tc.tile_pool(name="ob", bufs=3) as opool,
            tc.tile_pool(name="sm", bufs=8) as smpool,
            tc.tile_pool(name="ps_s", bufs=3, space="PSUM") as pspool,
            tc.tile_pool(name="ps_o", bufs=2, space="PSUM") as popool,
        ):
            # dummy exp up front: pulls the ScalarE exp table load (~2.7us)
            # off the first real ACTIVATE's critical path
            warm = kvpool.tile([128, 1], f32, tag="warm")
            nc.vector.memset(warm[:], 0.0)
            nc.scalar.activation(
                warm[:], warm[:], mybir.ActivationFunctionType.Exp, scale=1.0
            )

            # dummy matmul chain: ramps the PE HAM clock gate (1.2 -> 2.4 GHz
            # after ~3.4us of continuous work) while the first kT/qT DMAs are
            # still in flight; sized so it ends right as the first inputs
            # land (~10.5us) and the HAM window trips
            wlhs = kvpool.tile([128, 128], f16, tag="wlhs")
            wrhs = kvpool.tile([128, 256], f16, tag="wrhs")
            nc.vector.memset(wlhs[:], 0.0)
            nc.vector.memset(wrhs[:], 0.0)
            wps = pspool.tile([128, EB, QG], f32, tag="ps", name="warmps")
            for _ in range(11):
                nc.tensor.matmul(
                    wps[:, 0, 0:256], lhsT=wlhs[:], rhs=wrhs[:], start=True, stop=True
                )

            from collections import deque

            tri = kvpool.tile([128, 128], f16, tag="tri")
            kT = {}
            vaug = {}

            def _prep_b(b):
                kT_b = kvpool.tile([128, S], f16, tag=f"kT{b}", name=f"kT{b}")
                va = kvpool.tile([128, NT, 132], f16, tag=f"va{b}", name=f"va{b}")
                if b == 0:
                    # startup chunking: each group J's kT/va arrives just
                    # before its QK/PV batches need it, so the first real
                    # matmuls start as soon as the ring delivers ~96KB
                    nc.sync.dma_start(out=kT_b[:, 0:256], in_=k_in[b][:, 0:256])
                    nc.sync.dma_start(out=kT_b[:, 256:768], in_=k_in[b][:, 256:768])
                    nc.sync.dma_start(out=tri[:], in_=tri_in)
                    nc.sync.dma_start(out=va[:, 0:4, 0:128], in_=v_in[b][:, 0:4])
                    nc.sync.dma_start(out=kT_b[:, 768:S], in_=k_in[b][:, 768:S])
                    nc.sync.dma_start(out=va[:, 4:8, 0:128], in_=v_in[b][:, 4:8])
                    nc.sync.dma_start(out=va[:, 8:NT, 0:128], in_=v_in[b][:, 8:NT])
                else:
                    nc.sync.dma_start(out=kT_b[:], in_=k_in[b])
                    nc.sync.dma_start(out=va[:, :, 0:128], in_=v_in[b])
                kT[b] = kT_b
                nc.vector.memset(va[:, :, 128:129], 1.0)
                vaug[b] = va

            class Ctx:
                def __init__(self, b, h):
                    self.b, self.h = b, h
                    qT = qpool.tile([128, S], f16, tag="qT", name=f"qT{b}_{h}")
                    if (b, h) == (0, 0):
                        # first qT chunked on the Scalar ring so it loads in
                        # parallel with kT0 on the Sync ring (startup path)
                        nc.scalar.dma_start(out=qT[:, 0:QG], in_=q_in[b, h][:, 0:QG])
                        nc.scalar.dma_start(
                            out=qT[:, QG : 2 * QG], in_=q_in[b, h][:, QG : 2 * QG]
                        )
                        nc.scalar.dma_start(
                            out=qT[:, 2 * QG : S], in_=q_in[b, h][:, 2 * QG : S]
                        )
                    else:
                        nc.scalar.dma_start(out=qT[:], in_=q_in[b, h])
                    self.qT = qT
                    self.ob = opool.tile([128, NT, D], f16, tag="ob", name=f"ob{b}_{h}")
                    self.po_of = {}
                    self.defer = deque()  # deferred r>=2 PV batches
                    self.done_groups = 0
                    self.last = (b, h) == (B - 1, HPC - 1)
                    # reverse the group order on the final head so the tail
                    # after the last exp is the smallest group's work
                    Js = range(NG - 1, -1, -1) if self.last else range(NG)
                    self.batches = [
                        (J, ktl, qoff, eng)
                        for J in Js
                        for (ktl, qoff, eng) in _group_plan(J, self.last)
                    ]

                def norm_bank(self, J, x):
                    # batched: one reciprocal per po bank (2 q-rows), then a
                    # single broadcast multiply into the fp32 output tile
                    po = self.po_of[J][x]
                    linv = smpool.tile([128, 2, 1], f32, tag="linv", name="linv")
                    nc.vector.reciprocal(linv[:], po[:, 0:2, 128:129])
                    nc.vector.tensor_tensor(
                        self.ob[:, 4 * J + 2 * x : 4 * J + 2 * x + 2, :],
                        po[:, 0:2, 0:128],
                        linv[:].broadcast_to([128, 2, 128]),
                        mult,
                    )
                    if self.last:
                        # per-group stores (per-bank for the final group) so
                        # the tail's last DMA is small but store-issue time
                        # on the Sync queue stays off the critical path
                        if J == 0:
                            lo = 2 * x
                            nc.sync.dma_start(
                                out=out[self.b, self.h][:, lo : lo + 2, :],
                                in_=self.ob[:, lo : lo + 2, :],
                            )
                        elif x == 1:
                            nc.sync.dma_start(
                                out=out[self.b, self.h][:, 4 * J : 4 * J + 4, :],
                                in_=self.ob[:, 4 * J : 4 * J + 4, :],
                            )

                def emit_qk(self, J, ktl, qoff):
                    qw = QG - qoff
                    ps = pspool.tile([128, EB, qw], f32, tag="ps", name="ps")
                    pt = ptpool.tile([128, EB, qw], f16, tag="pt", name="pt")
                    for u, iu in enumerate(ktl):
                        # causal trim: k-tile iu only matters for queries
                        # >= 128*(iu-4J); the skipped region holds stale PSUM
                        # that exp bounds and PV never reads
                        qo = max(qoff, 128 * (iu - 4 * J))
                        nc.tensor.matmul(
                            ps[:, u, qo - qoff : qw],
                            lhsT=kT[self.b][:, ts(iu, 128)],
                            rhs=self.qT[:, J * QG + qo : (J + 1) * QG],
                            start=True,
                            stop=True,
                        )
                    return ps, pt

                def emit_exp(self, J, ktl, qoff, eng, ps, pt):
                    # stage 1: exp on the planned engine, then triangular
                    # masks for any diagonal tiles on the GpSimd engine
                    # (keeps DVE free and mask latency off the PV path)
                    nu = len(ktl)
                    if eng == "V":
                        nc.vector.tensor_scalar(
                            pt[:, 0:nu, :].bitcast(i16),
                            ps[:, 0:nu, :],
                            C1,
                            C2,
                            mult,
                            addop,
                        )
                    else:
                        nc.scalar.activation(
                            pt[:, 0:nu, :], ps[:, 0:nu, :], Exp, scale=SCALE
                        )
                    for u, iu in enumerate(ktl):
                        rp = iu - 4 * J
                        if rp >= 0:
                            lo = 128 * rp - qoff
                            nc.gpsimd.tensor_tensor(
                                pt[:, u, lo : lo + 128],
                                pt[:, u, lo : lo + 128],
                                tri[:],
                                mult,
                            )

                def emit_pv_stage(self, J, ktl, qoff, eng, ps, pt):
                    # stage 2: PV matmuls, PSUM-bank norms, stores
                    if J not in self.po_of:
                        # two packed PV accumulators: (r=0,1) and (r=2,3).
                        # The very last group's bank-1 comes out of the (by
                        # now draining) QK-score pool so the kernel tail does
                        # not serialize on the 2-buffer po rotation.
                        def _po(x):
                            if self.last and J == 0 and x == 1:
                                return pspool.tile(
                                    [128, 2, 132], f32, tag="ps", name="potail"
                                )
                            return popool.tile(
                                [128, 2, 132],
                                f32,
                                tag="po",
                                name=f"po{self.b}{self.h}{J}{x}",
                            )

                        self.po_of[J] = [_po(0), _po(1)]
                    # batches without diagonal tiles, plus J=0's diagonal
                    # batch (the head's first), defer their bank-1 PVs so the
                    # first bank-1 writes of a group land well after the
                    # previous group's bank-1 norm has freed the bank
                    if ktl[-1] < 4 * J or (J == 0 and ktl[-1] == 1):
                        for u, iu in enumerate(ktl):
                            self.emit_pv(J, qoff, pt, u, iu, 0, 2)
                            if iu - 4 * J == 1:
                                # bank 0 (r=0,1) completes with this batch:
                                # normalize early so it frees for the next
                                # group
                                self.norm_bank(J, 0)
                        self.defer.append((J, qoff, pt, ktl))
                        while len(self.defer) > 2:
                            self.flush_defer()
                        return
                    # diagonal/pair batch: flush this group's deferred bank-1
                    # accumulations, which must precede the immediate PVs
                    while self.defer:
                        self.flush_defer()
                    for u, iu in enumerate(ktl):
                        rp = iu - 4 * J
                        self.emit_pv(J, qoff, pt, u, iu, 0, 4)
                        if rp == 1:
                            self.norm_bank(J, 0)
                    if iu == 4 * J + 3:  # last batch of the group
                        self.norm_bank(J, 1)
                        self.done_groups += 1
                        self.store(J)

                def emit_pv(self, J, qoff, pt, u, iu, rlo, rhi):
                    rp = iu - 4 * J
                    po = self.po_of[J]
                    for r in range(max(rp, rlo), rhi):
                        # start=True clears has_written for the WHOLE bank;
                        # only the bank's first group (even r) may set it.
                        lo = 128 * r - qoff
                        nc.tensor.matmul(
                            po[r // 2][:, r % 2, 0:129],
                            lhsT=pt[:, u, lo : lo + 128],
                            rhs=vaug[self.b][:, iu, 0:129],
                            start=(iu == 0 and r % 2 == 0),
                            stop=(iu == 4 * J + r),
                        )

                def flush_defer(self):
                    J, qoff, pt, ktl = self.defer.popleft()
                    for u, iu in enumerate(ktl):
                        self.emit_pv(J, qoff, pt, u, iu, 2, 4)

                def store(self, J):
                    # stores go out the Sync ring (with kT/va loads), keeping
                    # the Scalar ring free for qT prefetches
                    if not self.last and self.done_groups == NG:
                        nc.sync.dma_start(
                            out=out[self.b, self.h], in_=self.ob[:]
                        )

            # one flat software-pipelined stream across all (b, h) with two
            # lag stages: batch n's QK matmuls are emitted at step n, its
            # exp+masks at step n+2 and its PV matmuls at step n+4, so the
            # in-order PE stream always has masked P ready when the PV
            # matmuls issue, including across head and sequence boundaries.
            heads = [(b, h) for b in range(B) for h in range(HPC)]
            _prep_b(0)
            pend = deque()  # QK emitted, awaiting exp+mask
            pend2 = deque()  # exp emitted, awaiting PV
            next_ctx = Ctx(*heads[0])

            def s1():
                item = pend.popleft()
                item[0].emit_exp(*item[1:])
                pend2.append(item)

            def s2():
                item = pend2.popleft()
                item[0].emit_pv_stage(*item[1:])

            for idx, (b, h) in enumerate(heads):
                ctx = next_ctx
                next_ctx = None
                if h == 0 and b + 1 < B:
                    _prep_b(b + 1)
                nbat = len(ctx.batches)
                for k, bt in enumerate(ctx.batches):
                    if nbat - k == 6 and idx + 1 < len(heads):
                        next_ctx = Ctx(*heads[idx + 1])
                    eb = ctx.emit_qk(bt[0], bt[1], bt[2])
                    pend.append((ctx, bt[0], bt[1], bt[2], bt[3], eb[0], eb[1]))
                    if len(pend2) > 2:
                        s2()
                    if len(pend) > 2:
                        s1()
                if next_ctx is None and idx + 1 < len(heads):
                    next_ctx = Ctx(*heads[idx + 1])
            while pend:
                if pend2:
                    s2()
                s1()
            while pend2:
                s2()
    nc.compile()
    return nc


def _get_nc():
    if "nc" not in _cache:
        _cache["nc"] = _build_nc()
    return _cache["nc"]


def _install_ntff_hook():
    """Register the axon NTFF profile hook that concourse expects under
    ``antenv.axon_hooks`` (the agent image lacks that module). Mirrors
    trn_agent_boot's ctypes shim. Returns True if profiling is available."""
    import contextlib
    import ctypes
    import types

    if "antenv.axon_hooks" in sys.modules:
        return True
    so_path = "/opt/axon/libaxon_pjrt.so"
    if not os.path.exists(so_path):
        return False
    lib = ctypes.CDLL(so_path)
    if not hasattr(lib, "axon_start_nrt_profile"):
        return False
    lib.axon_start_nrt_profile.argtypes = [
        ctypes.POINTER(ctypes.c_int64),
        ctypes.c_size_t,
    ]
    lib.axon_start_nrt_profile.restype = ctypes.c_int64
    lib.axon_stop_nrt_profile.argtypes = [ctypes.c_char_p]
    lib.axon_stop_nrt_profile.restype = ctypes.c_int64

    @contextlib.contextmanager
    def _hook(output_dir, device_ids):
        import jax

        jax.devices()
        if device_ids:
            ids = (ctypes.c_int64 * len(device_ids))(*device_ids)
            rc = lib.axon_start_nrt_profile(ids, len(device_ids))
        else:
            rc = lib.axon_start_nrt_profile(None, 0)
        if rc != 0:
            raise RuntimeError(f"axon_start_nrt_profile rc={rc}")
        try:
            yield
        finally:
            n = lib.axon_stop_nrt_profile(str(output_dir).encode())
            print(f"ntff profile: {n} file(s) -> {output_dir}", file=sys.stderr)

    import antenv

    mod = types.ModuleType("antenv.axon_hooks")
    _h = [_hook]
    mod.get_axon_ntff_profile_hook = lambda: _h[0]
    mod.set_axon_ntff_profile_hook = lambda h: _h.__setitem__(0, h)
    sys.modules["antenv.axon_hooks"] = mod
    antenv.axon_hooks = mod

    # keep the trace path local: no artifact upload from this container
    from concourse import bass_utils as _bu

    _bu.upload_artifacts = lambda d: f"file://{d}"
    return True


def kernel(q, k, v, k_cache, v_cache, slot_mapping, block_tables):
    global LAST_RESULTS
    from concourse.bass_utils import run_bass_kernel_spmd

    q = np.ascontiguousarray(np.asarray(q), dtype=np.float32)
    k = np.ascontiguousarray(np.asarray(k), dtype=np.float32)
    v = np.ascontiguousarray(np.asarray(v), dtype=np.float32)
    sm = np.asarray(slot_mapping).astype(np.int64)
    bt = np.asarray(block_tables).astype(np.int64)

    # paged KV-cache store + gather through block tables (host side: pure
    # data movement, mirrors the reference semantics incl. dropped slots)
    num_slots = NB * BS
    kc = np.asarray(k_cache, dtype=np.float32).reshape(num_slots, HKV, D).copy()
    vc = np.asarray(v_cache, dtype=np.float32).reshape(num_slots, HKV, D).copy()
    valid = (sm >= 0) & (sm < num_slots)
    kc[sm[valid]] = k[valid]
    vc[sm[valid]] = v[valid]
    btc = np.clip(bt, 0, NB - 1)  # jax gather clamps OOB indices
    k_seq = kc.reshape(NB, BS, HKV, D)[btc].reshape(B, S, HKV, D)
    v_seq = vc.reshape(NB, BS, HKV, D)[btc].reshape(B, S, HKV, D)

    # pre-transpose to [d, seq] so device DMA loads are plain contiguous
    q16 = q.reshape(B, S, H, D).astype(np.float16)
    qT = np.ascontiguousarray(q16.transpose(0, 2, 3, 1))  # [B, H, D, S]
    kT = np.ascontiguousarray(
        k_seq.astype(np.float16).transpose(0, 2, 3, 1)
    )  # [B, HKV, D, S]
    # v prepacked to [B, HKV, p, t, d] (s = t*128 + p) for contiguous DMA
    v16 = np.ascontiguousarray(
        v_seq.astype(np.float16).reshape(B, NT, 128, HKV, D).transpose(0, 3, 2, 1, 4)
    )
    tri = np.triu(np.ones((128, 128), dtype=np.float16))

    in_maps = []
    for c in range(NCORES):
        g = c // 2  # this core's KV head
        in_maps.append(
            {
                "q": np.ascontiguousarray(qT[:, HPC * c : HPC * (c + 1)]),
                "k": np.ascontiguousarray(kT[:, g]),
                "v": np.ascontiguousarray(v16[:, g]),
                "tri": tri,
            }
        )

    nc = _get_nc()
    trace = bool(int(os.environ.get("KERNEL_TRACE", "0")))
    if trace:
        trace = _install_ntff_hook()
    tmpdir = os.environ.get("KERNEL_TRACE_DIR") or None
    if tmpdir:
        os.makedirs(tmpdir, exist_ok=True)
    res = run_bass_kernel_spmd(
        nc, in_maps, core_ids=list(range(NCORES)), trace=trace, tmpdir=tmpdir
    )
    LAST_RESULTS = res

    out = np.empty((B, S, H, D), np.float32)
    for c in range(NCORES):
        # device layout [B, HPC, p, t, d] -> [B, s=t*128+p, HPC, d]
        r = res.results[c]["out"].astype(np.float32).transpose(0, 3, 2, 1, 4).reshape(B, S, HPC, D)
        out[:, :, HPC * c : HPC * (c + 1), :] = r
    return out.reshape(T, H, D)
